# revision 30
# baseline (speedup 1.0000x reference)
"""Trainium2 Bass kernel for nn_BiVision_VQA2 (B=64,T=32,D=768,N=901).

Data-parallel over batch: 8 batch elems per core x 8 cores.

Math simplifications (validated vs reference, numpy sim rel err ~4e-3):
  - ga/go attention have a single key token -> softmax==1 -> linear in cls;
    the (cls@W2+b2)@W3+b3 chains collapse to cls@M + c with M,c precomputed
    on the HOST.  The GRU input is constant over time, so its contribution
    wx = cls@(Mga@W_ih^T) + cw is one matmul.
  - local attention: scores = (qemb@W0+b0) @ W1_h^T / sqrt(dk) @ X^T with
    row-constant terms dropped; query pooling applied to the attention
    matrix before the @X contraction; value/out projections collapse to
    per-head M2 = W2_h@W3_h (host).
Precision strategy (validated in numpy, rel err 4.1e-3 < 2e-2):
  - GRU weights + hidden state in fp8e4m3 (x16 / x1 scaling), matmuls use
    DoubleRow perf mode.  Scores path fp8 (Qt x8, X x4).
  - Everything else bf16; host pre-transposes/packs all weights so the
    device does no weight reshaping.
Scheduling:
  - all weight DMAs emitted up-front (stream during the GRU)
  - GRU: merged psum tiles (r|z in one 3-bank tile), split sigmoids so
    they overlap the weight stream, h' = (1-z)*n + z*h with z-terms
    precomputed, n-path split so only the 2nd half is latency-exposed
  - phase D software-pipelined: softmax/value tail of batch b-1 is
    emitted after the score matmuls of batch b so the PE never stalls
  - occasional dummy matmuls keep the PE HAM clock-gate at 2.4 GHz
"""

import os
import numpy as np
import ml_dtypes
from contextlib import ExitStack

import concourse.bass as bass
import concourse.tile as tile
from concourse import bacc, mybir
from concourse.bass_utils import run_bass_kernel_spmd
from concourse.masks import make_identity

FP = mybir.dt.float32
BF = mybir.dt.bfloat16
F8 = mybir.dt.float8e4
OP = mybir.AluOpType
AF = mybir.ActivationFunctionType
DR = mybir.MatmulPerfMode.DoubleRow

NCORES = 8
BL = 8
D = 768
T = 32
G = 3 * D
NK = 900
NH = 2
DK = 384
ET = D // 128
SG = 16.0
SQ = 8.0
SX = 4.0
ESC = 1.0 / (SQ * SX * float(np.sqrt(DK)))

CH_NK = [(0, 512), (512, 388)]
CH_D = [(0, 512), (512, 256)]
KC = [(k, min(128, NK - k)) for k in range(0, NK, 128)]

KSTEPS = int(os.environ.get("KSTEPS", str(T)))
PHASES = int(os.environ.get("KPHASES", "4"))
NDUM = int(os.environ.get("KDUM", "0"))


def build():
    nc = bacc.Bacc("TRN2", target_bir_lowering=False, debug=False,
                   enable_asserts=False)

    clsT8_d = nc.dram_tensor("clsT8", [128, ET, 16], F8, kind="ExternalInput").ap()
    clsTb_d = nc.dram_tensor("clsTb", [128, ET, 16], BF, kind="ExternalInput").ap()
    mw8_d = nc.dram_tensor("mw8", [128, ET, G], F8, kind="ExternalInput").ap()
    cw8_d = nc.dram_tensor("cw8", [1, G], F8, kind="ExternalInput").ap()
    idext8_d = nc.dram_tensor("idext8", [128, 2, 16], F8, kind="ExternalInput").ap()
    extn8_d = nc.dram_tensor("extn8", [BL, D], F8, kind="ExternalInput").ap()
    wh8_d = nc.dram_tensor("wh8", [128, ET, G], F8, kind="ExternalInput").ap()
    h0t8_d = nc.dram_tensor("h0t8", [128, ET, 16], F8, kind="ExternalInput").ap()
    h0b_d = nc.dram_tensor("h0b", [BL, D], BF, kind="ExternalInput").ap()
    mg_d = nc.dram_tensor("mg", [128, ET, D], BF, kind="ExternalInput").ap()
    cgcol_d = nc.dram_tensor("cgcol", [128, ET], FP, kind="ExternalInput").ap()
    w0_d = nc.dram_tensor("w0", [128, ET, D], BF, kind="ExternalInput").ap()
    b0col_d = nc.dram_tensor("b0col", [128, ET], FP, kind="ExternalInput").ap()
    w1t_d = nc.dram_tensor("w1t", [128, ET, D], BF, kind="ExternalInput").ap()
    pmask_d = nc.dram_tensor("pmaskb", [64, 2], BF, kind="ExternalInput").ap()
    xn_d = nc.dram_tensor("xn", [BL, NK, D], BF, kind="ExternalInput").ap()
    xt8_d = nc.dram_tensor("xt8", [BL, D, 912], F8, kind="ExternalInput").ap()
    m2_d = nc.dram_tensor("m2", [128, 2 * ET, D], BF, kind="ExternalInput").ap()
    vconcol_d = nc.dram_tensor("vconcol", [128, ET], FP, kind="ExternalInput").ap()
    f1_d = nc.dram_tensor("f1", [128, 12, 1024], BF, kind="ExternalInput").ap()
    b1row_d = nc.dram_tensor("b1row", [1, 1024], BF, kind="ExternalInput").ap()
    f2_d = nc.dram_tensor("f2", [128, 8, 512], BF, kind="ExternalInput").ap()
    b2row_d = nc.dram_tensor("b2row", [1, 512], BF, kind="ExternalInput").ap()
    f3_d = nc.dram_tensor("f3", [128, 4, 1024], BF, kind="ExternalInput").ap()
    b3row_d = nc.dram_tensor("b3row", [1, 1024], BF, kind="ExternalInput").ap()
    out_d = nc.dram_tensor("out", [BL, 1024], FP, kind="ExternalOutput").ap()

    with tile.TileContext(nc) as tc, ExitStack() as ctx:
        cpool = ctx.enter_context(tc.tile_pool(name="const", bufs=1))
        tail = ctx.enter_context(tc.tile_pool(name="tail", bufs=1))
        psT = ctx.enter_context(tc.tile_pool(name="psT", bufs=1, space="PSUM"))

        identf = cpool.tile([128, 128], FP, tag="identf")
        make_identity(nc, identf[:])
        identb = cpool.tile([128, 128], BF, tag="identb")
        nc.vector.tensor_copy(identb[:], identf[:])
        ones8 = cpool.tile([1, 16], F8, tag="ones8")
        nc.vector.memset(ones8[:], 1.0)
        onesb = cpool.tile([1, 16], BF, tag="onesb")
        nc.vector.memset(onesb[:], 1.0)
        clsTb = cpool.tile([128, ET, 16], BF, tag="clsTb")
        nc.sync.dma_start(clsTb[:], clsTb_d[:])
        qembT = cpool.tile([128, ET, BL, T], BF, tag="qembT")
        goutT = cpool.tile([128, ET, BL], BF, tag="goutT")
        QtT8 = cpool.tile([128, ET, 512], F8, tag="QtT8")
        pcxT = cpool.tile([128, ET, 2 * BL], BF, tag="pcxT")
        wxn_sb = cpool.tile([BL, D], BF, tag="wxn_sb")
        pmaskb = cpool.tile([64, 2], BF, tag="pmaskb")
        nc.sync.dma_start(pmaskb[:], pmask_d[:])

        with tc.tile_pool(name="phW", bufs=1) as phW, \
             tc.tile_pool(name="pgru", bufs=1) as pgru, \
             tc.tile_pool(name="g1", bufs=2) as g1, \
             tc.tile_pool(name="phA", bufs=1) as phA:
            # --- critical-path DMAs first: phase A operands
            clsT8 = phA.tile([128, ET, 16], F8, tag="clsT8")
            nc.sync.dma_start(clsT8[:], clsT8_d[:])
            mw8 = phA.tile([128, ET, G], F8, tag="mw8")
            for (j0, jw) in [(0, 512), (512, 512), (1024, 512),
                             (1536, 512), (2048, 256)]:
                nc.sync.dma_start(mw8[:, :, j0:j0 + jw],
                                  mw8_d[:, :, j0:j0 + jw])
            cw8 = phA.tile([1, G], F8, tag="cw8")
            nc.sync.dma_start(cw8[:], cw8_d[:])
            # --- GRU weights (needed ~10us in)
            wh8 = pgru.tile([128, ET, G], F8, tag="wh8")
            for (j0, jw) in [(0, 512), (512, 512), (1024, 512),
                             (1536, 512), (2048, 256)]:
                nc.sync.dma_start(wh8[:, :, j0:j0 + jw],
                                  wh8_d[:, :, j0:j0 + jw])
            idext8 = pgru.tile([128, 2, 16], F8, tag="idext8")
            nc.sync.dma_start(idext8[:], idext8_d[:])
            ext8 = pgru.tile([128, 2, G], F8, tag="ext8")
            nc.vector.memset(ext8[:].rearrange("p a b -> p (a b)"), 0.0)
            nc.sync.dma_start(ext8[0:BL, 0, 2 * D:3 * D], extn8_d[:])
            # --- GRU initial state (needed immediately after phase A)
            hT8s = [pgru.tile([128, ET, 16], F8, tag=f"hT8{i}",
                              name=f"hT8{i}") for i in range(2)]
            nc.sync.dma_start(hT8s[0][:], h0t8_d[:])
            hnat0 = g1.tile([BL, D], BF, tag="hnat")
            nc.sync.dma_start(hnat0[:], h0b_d[:])
            # --- mid/late-phase weight tiles (DMAs emitted after phase A)
            mg = phW.tile([128, ET, D], BF, tag="mg")
            cgcol = phW.tile([128, ET], FP, tag="cgcol")
            w0 = phW.tile([128, ET, D], BF, tag="w0")
            b0col = phW.tile([128, ET], FP, tag="b0col")
            w1tt = phW.tile([128, ET, D], BF, tag="w1t")
            m2 = tail.tile([128, 2 * ET, D], BF, tag="m2")
            vconcol = tail.tile([128, ET], FP, tag="vconcol")
            f1 = tail.tile([128, 12, 1024], BF, tag="f1")
            f2 = tail.tile([128, 8, 512], BF, tag="f2")
            f3 = tail.tile([128, 4, 1024], BF, tag="f3")
            b1row = tail.tile([1, 1024], BF, tag="b1row")
            b2row = tail.tile([1, 512], BF, tag="b2row")
            b3row = tail.tile([1, 1024], BF, tag="b3row")

            with tc.tile_pool(name="psG", bufs=1, space="PSUM") as psG:
                # ============ phase A: wx = cls@MW + cw ====================

                def gate_mms(lhsT, rhs_w, prz, pn):
                    """emit the 20 DR matmuls for one full [8, 2304] gate set"""
                    subs = [(prz, 0, 512, 0), (prz, 512, 512, 512),
                            (prz, 1024, 512, 1024), (pn, 0, 512, 1536),
                            (pn, 512, 256, 2048)]
                    out = []
                    for (dst, o0, w_, j0) in subs:
                        for g in range(3):
                            nc.tensor.matmul(dst[:, o0:o0 + w_],
                                             lhsT[:, 2 * g:2 * g + 2, :],
                                             rhs_w[:, 2 * g:2 * g + 2, j0:j0 + w_],
                                             start=(g == 0), stop=False,
                                             perf_mode=DR)
                        out.append((dst, o0, w_, j0))
                    return out

                przA = psG.tile([16, 1536], FP, tag="prz")
                pnA = psG.tile([16, 768], FP, tag="pn")
                for (dst, o0, w_, j0) in gate_mms(clsT8, mw8, przA, pnA):
                    nc.tensor.matmul(dst[:, o0:o0 + w_], ones8[:1, :],
                                     cw8[:, j0:j0 + w_], start=False, stop=True)
                nc.scalar.activation(ext8[0:BL, 0, 0:1536], przA[:BL, :],
                                     AF.Copy)
                nc.scalar.activation(wxn_sb[:], pnA[:BL, :], AF.Copy)


                # ============ phase B: GRU =================================
                nc.gpsimd.memset(hT8s[1][:, :, BL:16], 0.0)
                hnat = hnat0

                SUBS = [(0, 0, 512, 0), (0, 512, 512, 512),
                        (0, 1024, 512, 1024), (1, 0, 512, 1536),
                        (1, 512, 256, 2048)]

                def emit_ext(prz_, pn_):
                    """pre-fill next step's psum with the wx/bias rows; no
                    dependency on the recurrent state, so these fill the PE
                    idle gap during the gate-math tail"""
                    for (ti, o0, w__, j0) in SUBS:
                        dst = prz_ if ti == 0 else pn_
                        nc.tensor.matmul(dst[:, o0:o0 + w__], idext8[:],
                                         ext8[:, :, j0:j0 + w__],
                                         start=True, stop=False, perf_mode=DR)

                prz = psG.tile([16, 1536], FP, tag="prz", name="prz_p0")
                pn = psG.tile([16, 768], FP, tag="pn", name="pn_p0")
                emit_ext(prz, pn)

                for t in range(KSTEPS):
                    hT8 = hT8s[t % 2]
                    hT8n = hT8s[(t + 1) % 2]
                    subs = [(prz, 0, 512, 0), (prz, 512, 512, 512),
                            (prz, 1024, 512, 1024), (pn, 0, 512, 1536),
                            (pn, 512, 256, 2048)]
                    rz = g1.tile([BL, 2 * D], BF, tag="rz")
                    u_ = g1.tile([BL, D], BF, tag="u")
                    w_ = g1.tile([BL, D], BF, tag="w")
                    t1 = g1.tile([BL, D], BF, tag="t1")
                    t2 = g1.tile([BL, D], BF, tag="t2")
                    nt_ = g1.tile([BL, D], BF, tag="nt")
                    v_ = g1.tile([BL, D], BF, tag="v")
                    hnew = g1.tile([BL, D], BF, tag="hnat")

                    def sub_mms(si):
                        dst, o0, w__, j0 = subs[si]
                        for g in range(3):
                            nc.tensor.matmul(dst[:, o0:o0 + w__],
                                             hT8[:, 2 * g:2 * g + 2, :],
                                             wh8[:, 2 * g:2 * g + 2, j0:j0 + w__],
                                             start=False, stop=(g == 2),
                                             perf_mode=DR)

                    sub_mms(0)
                    sub_mms(1)
                    # r ready -> sigmoid overlaps remaining stream
                    nc.scalar.activation(rz[:, 0:D], prz[:BL, 0:D], AF.Sigmoid,
                                         scale=1.0 / SG)
                    sub_mms(2)
                    nc.scalar.activation(rz[:, D:2 * D], prz[:BL, D:2 * D],
                                         AF.Sigmoid, scale=1.0 / SG)
                    # z-dependent terms during the n-chunk stream
                    nc.vector.tensor_mul(u_[:], rz[:, D:2 * D], hnat[:])
                    nc.vector.tensor_scalar(w_[:], rz[:, D:2 * D], -1.0, 1.0,
                                            OP.mult, OP.add)
                    sub_mms(3)
                    nc.vector.tensor_mul(t1[:, 0:512], rz[:, 0:512],
                                         pn[:BL, 0:512])
                    nc.vector.tensor_add(t2[:, 0:512], t1[:, 0:512],
                                         wxn_sb[:, 0:512])
                    nc.scalar.activation(nt_[:, 0:512], t2[:, 0:512], AF.Tanh,
                                         scale=1.0 / SG)
                    sub_mms(4)
                    for dd in range(NDUM):
                        pdum = psT.tile([16, 512], FP, tag="pt",
                                        padded_shape=[128, 1024])
                        nc.tensor.matmul(pdum[:], idext8[:],
                                         ext8[:, :, 0:512], start=True,
                                         stop=True, perf_mode=DR)
                    nc.vector.tensor_mul(t1[:, 512:768], rz[:, 512:768],
                                         pn[:BL, 512:768])
                    nc.vector.tensor_add(t2[:, 512:768], t1[:, 512:768],
                                         wxn_sb[:, 512:768])
                    nc.scalar.activation(nt_[:, 512:768], t2[:, 512:768],
                                         AF.Tanh, scale=1.0 / SG)
                    if t < KSTEPS - 1:
                        prz = psG.tile([16, 1536], FP, tag="prz",
                                       name=f"prz_p{t + 1}")
                        pn = psG.tile([16, 768], FP, tag="pn",
                                      name=f"pn_p{t + 1}")
                        emit_ext(prz, pn)
                    nc.vector.tensor_mul(v_[:], w_[:], nt_[:])
                    nc.vector.tensor_add(hnew[:], v_[:], u_[:])
                    pt = psT.tile([128, 64], BF, tag="pt",
                                  padded_shape=[128, 1024])
                    for kt in range(ET):
                        nc.tensor.matmul(pt[:, 8 * kt:8 * kt + 8],
                                         hnew[:, 128 * kt:128 * (kt + 1)],
                                         identb[:BL, :BL], is_transpose=True,
                                         skip_group_check=True)
                    nc.vector.tensor_copy(
                        hT8n[:, :, 0:BL],
                        pt[:, :48].rearrange("p (a b) -> p a b", b=BL))
                    nc.scalar.copy(
                        qembT[:, :, :, t].rearrange("p a b -> p (a b)"),
                        pt[:, :48])
                    hnat = hnew

                    if t == 14:
                        gate1 = g1.tile([1, BL], BF, tag="gate1")
                        nc.sync.dma_start(gate1[:], qembT[0:1, 0, :, 14])
                        for tt in range(0, ET, 3):
                            nc.sync.dma_start(mg[:, tt:tt + 3, :],
                                              mg_d[:, tt:tt + 3, :])
                        nc.sync.dma_start(cgcol[:], cgcol_d[:])
                        for tt in range(0, ET, 3):
                            nc.sync.dma_start(w0[:, tt:tt + 3, :],
                                              w0_d[:, tt:tt + 3, :])
                        nc.sync.dma_start(b0col[:], b0col_d[:])
                        for tt in range(0, ET, 3):
                            nc.sync.dma_start(w1tt[:, tt:tt + 3, :],
                                              w1t_d[:, tt:tt + 3, :])
                    if t == 22:
                        gate2 = g1.tile([1, BL], BF, tag="gate2")
                        nc.sync.dma_start(gate2[:], qembT[0:1, 0, :, 22])
                        for tt in range(0, 2 * ET, 3):
                            nc.sync.dma_start(m2[:, tt:tt + 3, :],
                                              m2_d[:, tt:tt + 3, :])
                        nc.sync.dma_start(vconcol[:], vconcol_d[:])
                        for tt in range(0, 12, 3):
                            nc.sync.dma_start(f1[:, tt:tt + 3, :],
                                              f1_d[:, tt:tt + 3, :])
                        for tt in range(0, 8, 4):
                            nc.sync.dma_start(f2[:, tt:tt + 4, :],
                                              f2_d[:, tt:tt + 4, :])
                        for tt in range(0, 4, 2):
                            nc.sync.dma_start(f3[:, tt:tt + 2, :],
                                              f3_d[:, tt:tt + 2, :])
                        nc.sync.dma_start(b1row[:], b1row_d[:])
                        nc.sync.dma_start(b2row[:], b2row_d[:])
                        nc.sync.dma_start(b3row[:], b3row_d[:])

            # ============ gout = cls@Mg + cg (feature-major) ==============
            with tc.tile_pool(name="psC", bufs=2, space="PSUM") as psC:
                for mt in range(ET):
                    p = psC.tile([128, BL], FP, tag="pg",
                                 padded_shape=[128, 512])
                    for kt in range(ET):
                        nc.tensor.matmul(p[:],
                                         mg[:, kt, 128 * mt:128 * (mt + 1)],
                                         clsTb[:, kt, 0:BL],
                                         start=(kt == 0), stop=(kt == ET - 1))
                    nc.vector.tensor_scalar(goutT[:, mt, :], p[:],
                                            cgcol[:, mt:mt + 1], None, OP.add)

                # ============ phase C: QT, QtT8 ===========================
                if PHASES >= 2:
                    qflat = qembT[:].rearrange("p a b t -> p a (b t)")
                    QT = g1.tile([128, ET, BL * T], BF, tag="QT")
                    for mt in range(ET):
                        p = psC.tile([128, BL * T], FP, tag="pc",
                                     padded_shape=[128, 512])
                        for kt in range(ET):
                            nc.tensor.matmul(
                                p[:], w0[:, kt, 128 * mt:128 * (mt + 1)],
                                qflat[:, kt, :],
                                start=(kt == 0), stop=(kt == ET - 1))
                        nc.vector.tensor_scalar(QT[:, mt, :], p[:],
                                                b0col[:, mt:mt + 1], None,
                                                OP.add)
                    for hd in range(NH):
                        for mt in range(ET):
                            p = psC.tile([128, BL * T], FP, tag="pc",
                                         padded_shape=[128, 512])
                            for i in range(3):
                                kt = 3 * hd + i
                                nc.tensor.matmul(
                                    p[:], w1tt[:, kt, 128 * mt:128 * (mt + 1)],
                                    QT[:, kt, :],
                                    start=(i == 0), stop=(i == 2))
                            dst = QtT8[:, mt, :].rearrange(
                                "p (b h2 t) -> p b h2 t", h2=NH, t=T)[:, :, hd, :]
                            src = p[:].rearrange("p (b t) -> p b t", t=T)
                            nc.scalar.activation(dst, src, AF.Copy, scale=SQ)

        # ================= phase D: per-b attention (sw-pipelined) ========
        if PHASES >= 3:
            with tc.tile_pool(name="xb", bufs=3) as xb, \
                 tc.tile_pool(name="ab", bufs=2) as ab, \
                 tc.tile_pool(name="psS", bufs=2, space="PSUM") as psS:
                state = {}
                gate3 = ab.tile([1, BL], BF, tag="gate3")
                nc.gpsimd.dma_start(gate3[:], qembT[0:1, 0, :, 24])

                def emit_scores(b):
                    xn_t = xb.tile([128, 8, D], BF, tag="xn")
                    nc.gpsimd.memset(xn_t[:, 7, :], 0.0)
                    xsrc = xn_d[b, 0:896, :].rearrange("(c p) d -> p c d", p=128)
                    nc.gpsimd.dma_start(xn_t[:, 0:4, :], xsrc[:, 0:4, :])
                    nc.gpsimd.dma_start(xn_t[:, 4:7, :], xsrc[:, 4:7, :])
                    nc.gpsimd.dma_start(xn_t[:4, 7, :], xn_d[b, 896:900, :])
                    xt_t = xb.tile([128, ET, 912], F8, tag="xt")
                    tsrc = xt8_d[b, :, :].rearrange("(c p) n -> p c n", p=128)
                    nc.gpsimd.dma_start(xt_t[:, 0:3, :], tsrc[:, 0:3, :])
                    nc.gpsimd.dma_start(xt_t[:, 3:6, :], tsrc[:, 3:6, :])
                    att = ab.tile([64, NK], BF, tag="att")
                    zacc = ab.tile([64, 2], FP, tag="zacc")
                    for ci, (n0, nw) in enumerate(CH_NK):
                        p = psS.tile([64, 512], FP, tag="s")
                        for g in range(3):
                            nc.tensor.matmul(
                                p[:, :nw],
                                QtT8[:, 2 * g:2 * g + 2, 64 * b:64 * b + 64],
                                xt_t[:, 2 * g:2 * g + 2, n0:n0 + nw],
                                start=(g == 0), stop=(g == 2), perf_mode=DR)
                        nc.scalar.activation(att[:, n0:n0 + nw], p[:, :nw],
                                             AF.Exp, scale=ESC,
                                             accum_out=zacc[:, ci:ci + 1])
                    state[b] = (xn_t, att, zacc)

                def emit_tail(b):
                    xn_t, att, zacc = state.pop(b)
                    zs = ab.tile([64, 1], FP, tag="zs")
                    nc.vector.tensor_add(zs[:], zacc[:, 0:1], zacc[:, 1:2])
                    rcp = ab.tile([64, 1], FP, tag="rcp")
                    nc.vector.reciprocal(rcp[:], zs[:])
                    wm = ab.tile([64, 2], BF, tag="wm")
                    nc.vector.tensor_scalar(wm[:], pmaskb[:], rcp[:, 0:1],
                                            None, OP.mult)
                    pa_sb = ab.tile([2, NK], BF, tag="pa_sb")
                    for ci, (n0, nw) in enumerate(CH_NK):
                        p2 = psS.tile([2, 512], FP, tag="p2")
                        nc.tensor.matmul(p2[:, :nw], wm[:], att[:, n0:n0 + nw],
                                         start=True, stop=True)
                        nc.vector.tensor_copy(pa_sb[:, n0:n0 + nw],
                                              p2[:, :nw])
                    paT = ab.tile([128, len(KC), 2], BF, tag="paT")
                    nc.gpsimd.memset(paT[:].rearrange("p a b -> p (a b)"), 0.0)
                    ptp = psT.tile([128, 16], BF, tag="pt",
                                   padded_shape=[128, 1024])
                    for c, (k0, kw) in enumerate(KC):
                        nc.tensor.matmul(ptp[:kw, 2 * c:2 * c + 2],
                                         pa_sb[:, k0:k0 + kw], identb[:2, :2],
                                         is_transpose=True,
                                         skip_group_check=True)
                        nc.vector.tensor_copy(paT[:kw, c, :],
                                              ptp[:kw, 2 * c:2 * c + 2])
                    pcx = ab.tile([2, D], BF, tag="pcx")
                    for ci, (n0, nw) in enumerate(CH_D):
                        p = psS.tile([2, 512], FP, tag="v")
                        for c in range(len(KC)):
                            nc.tensor.matmul(p[:, :nw], paT[:, c, :],
                                             xn_t[:, c, n0:n0 + nw],
                                             start=(c == 0),
                                             stop=(c == len(KC) - 1))
                        nc.vector.tensor_copy(pcx[:, n0:n0 + nw], p[:, :nw])
                    ptc = psT.tile([128, 16], BF, tag="pt",
                                   padded_shape=[128, 1024])
                    for kt in range(ET):
                        nc.tensor.matmul(ptc[:, 2 * kt:2 * kt + 2],
                                         pcx[:, 128 * kt:128 * (kt + 1)],
                                         identb[:2, :2], is_transpose=True,
                                         skip_group_check=True)
                    nc.vector.tensor_copy(
                        pcxT[:, :, 2 * b:2 * b + 2],
                        ptc[:, :2 * ET].rearrange("p (a c) -> p a c", c=2))

                for b in range(BL):
                    emit_scores(b)
                    if b > 0:
                        emit_tail(b - 1)
                emit_tail(BL - 1)

        # ================= phase E: projections + MLP =====================
        if PHASES >= 4:
            with tc.tile_pool(name="psE", bufs=2, space="PSUM") as psE:
                loT = tail.tile([128, ET, BL], BF, tag="loT")
                for mt in range(ET):
                    p = psE.tile([128, BL], FP, tag="pe",
                                 padded_shape=[128, 512])
                    k = 0
                    for hd in range(NH):
                        for kt in range(ET):
                            pcv = pcxT[:].rearrange(
                                "p a (b h) -> p a b h", h=2)[:, kt, :, hd]
                            nc.tensor.matmul(
                                p[:], m2[:, ET * hd + kt, 128 * mt:128 * (mt + 1)],
                                pcv, start=(k == 0), stop=(k == 2 * ET - 1))
                            k += 1
                    nc.vector.tensor_scalar(loT[:, mt, :], p[:],
                                            vconcol[:, mt:mt + 1], None,
                                            OP.add)

                y1b = tail.tile([BL, 1024], BF, tag="y1b")
                for ch in range(2):
                    p = psE.tile([BL, 512], FP, tag="pe")
                    for kt in range(12):
                        lhs = loT[:, kt, :] if kt < ET else goutT[:, kt - ET, :]
                        nc.tensor.matmul(p[:], lhs,
                                         f1[:, kt, 512 * ch:512 * (ch + 1)],
                                         start=(kt == 0), stop=False)
                    nc.tensor.matmul(p[:], onesb[:1, :BL],
                                     b1row[:, 512 * ch:512 * (ch + 1)],
                                     start=False, stop=True)
                    nc.scalar.activation(y1b[:, 512 * ch:512 * (ch + 1)], p[:],
                                         AF.Copy)
                pt1 = psT.tile([128, 64], BF, tag="pt",
                               padded_shape=[128, 1024])
                for kt in range(8):
                    nc.tensor.matmul(pt1[:, 8 * kt:8 * kt + 8],
                                     y1b[:, 128 * kt:128 * (kt + 1)],
                                     identb[:BL, :BL], is_transpose=True,
                                     skip_group_check=True)
                y1T = tail.tile([128, 8, BL], BF, tag="y1T")
                nc.vector.tensor_copy(y1T[:].rearrange("p a b -> p (a b)"),
                                      pt1[:, :64])

                p = psE.tile([BL, 512], FP, tag="pe")
                for kt in range(8):
                    nc.tensor.matmul(p[:], y1T[:, kt, :], f2[:, kt, :],
                                     start=(kt == 0), stop=False)
                nc.tensor.matmul(p[:], onesb[:1, :BL], b2row[:],
                                 start=False, stop=True)
                y2b = tail.tile([BL, 512], BF, tag="y2b")
                nc.scalar.activation(y2b[:], p[:], AF.Relu)
                pt2 = psT.tile([128, 32], BF, tag="pt",
                               padded_shape=[128, 1024])
                for kt in range(4):
                    nc.tensor.matmul(pt2[:, 8 * kt:8 * kt + 8],
                                     y2b[:, 128 * kt:128 * (kt + 1)],
                                     identb[:BL, :BL], is_transpose=True,
                                     skip_group_check=True)
                y2T = tail.tile([128, 4, BL], BF, tag="y2T")
                nc.vector.tensor_copy(y2T[:].rearrange("p a b -> p (a b)"),
                                      pt2[:, :32])

                ynat = tail.tile([BL, 1024], FP, tag="ynat")
                for ch in range(2):
                    p = psE.tile([BL, 512], FP, tag="pe")
                    for kt in range(4):
                        nc.tensor.matmul(p[:], y2T[:, kt, :],
                                         f3[:, kt, 512 * ch:512 * (ch + 1)],
                                         start=(kt == 0), stop=False)
                    nc.tensor.matmul(p[:], onesb[:1, :BL],
                                     b3row[:, 512 * ch:512 * (ch + 1)],
                                     start=False, stop=True)
                    nc.vector.tensor_copy(ynat[:, 512 * ch:512 * (ch + 1)],
                                          p[:])
                nc.sync.dma_start(out_d[:, :], ynat[:])

    nc.compile()
    return nc


_NC = None


def _bf(x):
    return np.ascontiguousarray(x).astype(ml_dtypes.bfloat16)


def _f8(x):
    return np.ascontiguousarray(x).astype(ml_dtypes.float8_e4m3)


def _tile6(w):
    """[768, J] -> [128, 6, J] with [p, t, j] = w[128t+p, j]"""
    J = w.shape[1]
    return np.ascontiguousarray(w.reshape(ET, 128, J).transpose(1, 0, 2))


def make_in_maps(inputs):
    f32 = np.float32
    img = np.asarray(inputs["image_local_embeds"], f32)
    h0 = np.asarray(inputs["h0"], f32)
    w_ih = np.asarray(inputs["gru_w_ih"], f32)
    w_hh = np.asarray(inputs["gru_w_hh"], f32)
    b_ih = np.asarray(inputs["gru_b_ih"], f32)
    b_hh = np.asarray(inputs["gru_b_hh"], f32)
    ga_w = np.asarray(inputs["ga_w"], f32)
    ga_b = np.asarray(inputs["ga_b"], f32)
    ga_pool = np.asarray(inputs["ga_pool"], f32)
    la_w = np.asarray(inputs["la_w"], f32)
    la_b = np.asarray(inputs["la_b"], f32)
    la_pool = np.asarray(inputs["la_pool"], f32)
    go_w = np.asarray(inputs["go_w"], f32)
    go_b = np.asarray(inputs["go_b"], f32)
    go_pool = np.asarray(inputs["go_pool"], f32)
    f1_w = np.asarray(inputs["f1_w"], f32)
    f1_b = np.asarray(inputs["f1_b"], f32)
    f2_w = np.asarray(inputs["f2_w"], f32)
    f2_b = np.asarray(inputs["f2_b"], f32)
    f3_w = np.asarray(inputs["f3_w"], f32)
    f3_b = np.asarray(inputs["f3_b"], f32)

    Mga = ga_pool[0] * (ga_w[2] @ ga_w[3])
    cga = ga_pool[0] * (ga_b[2] @ ga_w[3] + ga_b[3])
    MW = Mga @ w_ih.T
    cw = cga @ w_ih.T + b_ih
    cw[:2 * D] += b_hh[:2 * D]
    Sgo = go_pool.sum()
    Mg = Sgo * (go_w[2] @ go_w[3])
    cg = Sgo * (go_b[2] @ go_w[3] + go_b[3])
    Sla = la_pool.sum()
    M2 = np.stack([la_w[2][:, hd * DK:(hd + 1) * DK]
                   @ la_w[3][hd * DK:(hd + 1) * DK, :] for hd in range(NH)])
    vcon = Sla * (la_b[2] @ la_w[3] + la_b[3])
    W1T = np.ascontiguousarray(la_w[1].T)

    idext = np.zeros((128, 2, 16), f32)
    for b in range(BL):
        idext[b, 0, b] = 1.0
    pmask = np.zeros((64, 2), f32)
    pmask[0:T, 0] = la_pool
    pmask[T:2 * T, 1] = la_pool

    mw8 = _f8(_tile6(SG * MW))
    cw8 = _f8((SG * cw)[None, :])
    wh8 = _f8(_tile6(SG * w_hh.T))
    extn8 = _f8(np.broadcast_to(SG * b_hh[2 * D:], (BL, D)).copy())
    idext8 = _f8(idext)
    mg = _bf(_tile6(Mg))
    cgcol = np.ascontiguousarray(cg.reshape(ET, 128).T)
    w0 = _bf(_tile6(la_w[0]))
    b0col = np.ascontiguousarray(la_b[0].reshape(ET, 128).T)
    w1t = _bf(_tile6(W1T))
    m2 = _bf(np.concatenate([_tile6(M2[0]), _tile6(M2[1])], axis=1))
    vconcol = np.ascontiguousarray(vcon.reshape(ET, 128).T)
    f1p = _bf(f1_w.reshape(12, 128, 1024).transpose(1, 0, 2))
    f2p = _bf(f2_w.reshape(8, 128, 512).transpose(1, 0, 2))
    f3p = _bf(f3_w.reshape(4, 128, 1024).transpose(1, 0, 2))

    in_maps = []
    B = img.shape[0]
    per = B // NCORES
    for c in range(NCORES):
        sl = slice(c * per, (c + 1) * per)
        cls = img[sl, 0, :]
        X = img[sl, 1:, :]
        clsT = np.zeros((128, ET, 16), f32)
        clsT[:, :, :BL] = cls.T.reshape(ET, 128, BL).transpose(1, 0, 2)
        h0c = h0[sl]
        h0t = np.zeros((128, ET, 16), f32)
        h0t[:, :, :BL] = h0c.T.reshape(ET, 128, BL).transpose(1, 0, 2)
        xt = np.zeros((per, D, 912), f32)
        xt[:, :, :NK] = SX * X.transpose(0, 2, 1)
        m = {
            "clsT8": _f8(clsT),
            "clsTb": _bf(clsT),
            "mw8": mw8, "cw8": cw8, "idext8": idext8, "extn8": extn8,
            "wh8": wh8,
            "h0t8": _f8(h0t), "h0b": _bf(h0c),
            "mg": mg, "cgcol": cgcol.astype(f32),
            "w0": w0, "b0col": b0col.astype(f32),
            "w1t": w1t, "pmaskb": _bf(pmask),
            "xn": _bf(X), "xt8": _f8(xt),
            "m2": m2, "vconcol": vconcol.astype(f32),
            "f1": f1p, "b1row": _bf(f1_b[None, :]),
            "f2": f2p, "b2row": _bf(f2_b[None, :]),
            "f3": f3p, "b3row": _bf(f3_b[None, :]),
        }
        in_maps.append(m)
    return in_maps


def kernel(**inputs):
    global _NC
    if _NC is None:
        _NC = build()
    in_maps = make_in_maps(inputs)
    res = run_bass_kernel_spmd(_NC, in_maps, core_ids=list(range(NCORES)))
    return np.concatenate([res.results[c]["out"] for c in range(NCORES)],
                          axis=0)


# revision 32
# speedup vs baseline: 1.0811x; 1.0811x over previous
"""Trainium2 Bass kernel for nn_BiVision_VQA2 (B=64,T=32,D=768,N=901).

Data-parallel over batch: 8 batch elems per core x 8 cores.

Math simplifications (validated vs reference, numpy sim rel err ~4e-3):
  - ga/go attention have a single key token -> softmax==1 -> linear in cls;
    the (cls@W2+b2)@W3+b3 chains collapse to cls@M + c with M,c precomputed
    on the HOST.  The GRU input is constant over time, so its contribution
    wx = cls@(Mga@W_ih^T) + cw is one matmul.
  - local attention: scores = (qemb@W0+b0) @ W1_h^T / sqrt(dk) @ X^T with
    row-constant terms dropped; query pooling applied to the attention
    matrix before the @X contraction; value/out projections collapse to
    per-head M2 = W2_h@W3_h (host).
Precision strategy (validated in numpy, rel err 4.1e-3 < 2e-2):
  - GRU weights + hidden state in fp8e4m3 (x16 / x1 scaling), matmuls use
    DoubleRow perf mode.  Scores path fp8 (Qt x8, X x4).
  - Everything else bf16; host pre-transposes/packs all weights so the
    device does no weight reshaping.
Scheduling:
  - all weight DMAs emitted up-front (stream during the GRU)
  - GRU: merged psum tiles (r|z in one 3-bank tile), split sigmoids so
    they overlap the weight stream, h' = (1-z)*n + z*h with z-terms
    precomputed, n-path split so only the 2nd half is latency-exposed
  - phase D software-pipelined: softmax/value tail of batch b-1 is
    emitted after the score matmuls of batch b so the PE never stalls
  - occasional dummy matmuls keep the PE HAM clock-gate at 2.4 GHz
"""

import os
import numpy as np
import ml_dtypes
from contextlib import ExitStack

import concourse.bass as bass
import concourse.tile as tile
from concourse import bacc, mybir
from concourse.bass_utils import run_bass_kernel_spmd
from concourse.masks import make_identity

FP = mybir.dt.float32
BF = mybir.dt.bfloat16
F8 = mybir.dt.float8e4
OP = mybir.AluOpType
AF = mybir.ActivationFunctionType
DR = mybir.MatmulPerfMode.DoubleRow

NCORES = 8
BL = 8
D = 768
T = 32
G = 3 * D
NK = 900
NH = 2
DK = 384
ET = D // 128
SG = 16.0
SQ = 8.0
SX = 4.0
ESC = 1.0 / (SQ * SX * float(np.sqrt(DK)))

CH_NK = [(0, 512), (512, 388)]
CH_D = [(0, 512), (512, 256)]
KC = [(k, min(128, NK - k)) for k in range(0, NK, 128)]

KSTEPS = int(os.environ.get("KSTEPS", str(T)))
PHASES = int(os.environ.get("KPHASES", "4"))
NDUM = int(os.environ.get("KDUM", "0"))


def build():
    nc = bacc.Bacc("TRN2", target_bir_lowering=False, debug=False,
                   enable_asserts=False)

    clsT8_d = nc.dram_tensor("clsT8", [128, ET, 16], F8, kind="ExternalInput").ap()
    clsTb_d = nc.dram_tensor("clsTb", [128, ET, 16], BF, kind="ExternalInput").ap()
    mw8_d = nc.dram_tensor("mw8", [128, ET, G], F8, kind="ExternalInput").ap()
    cw8_d = nc.dram_tensor("cw8", [1, G], F8, kind="ExternalInput").ap()
    idext8_d = nc.dram_tensor("idext8", [128, 2, 16], F8, kind="ExternalInput").ap()
    extn8_d = nc.dram_tensor("extn8", [BL, D], F8, kind="ExternalInput").ap()
    wh8_d = nc.dram_tensor("wh8", [128, ET, G], F8, kind="ExternalInput").ap()
    h0t8_d = nc.dram_tensor("h0t8", [128, ET, 16], F8, kind="ExternalInput").ap()
    h0b_d = nc.dram_tensor("h0b", [BL, D], BF, kind="ExternalInput").ap()
    mg_d = nc.dram_tensor("mg", [128, ET, D], BF, kind="ExternalInput").ap()
    cgcol_d = nc.dram_tensor("cgcol", [128, ET], FP, kind="ExternalInput").ap()
    w0_d = nc.dram_tensor("w0", [128, ET, D], BF, kind="ExternalInput").ap()
    b0col_d = nc.dram_tensor("b0col", [128, ET], FP, kind="ExternalInput").ap()
    w1t_d = nc.dram_tensor("w1t", [128, ET, D], BF, kind="ExternalInput").ap()
    pmask_d = nc.dram_tensor("pmaskb", [64, 2], BF, kind="ExternalInput").ap()
    xn_d = nc.dram_tensor("xn", [BL, NK, D], BF, kind="ExternalInput").ap()
    xt8_d = nc.dram_tensor("xt8", [BL, D, 912], F8, kind="ExternalInput").ap()
    m2_d = nc.dram_tensor("m2", [128, 2 * ET, D], BF, kind="ExternalInput").ap()
    vconcol_d = nc.dram_tensor("vconcol", [128, ET], FP, kind="ExternalInput").ap()
    f1_d = nc.dram_tensor("f1", [128, 12, 1024], BF, kind="ExternalInput").ap()
    b1row_d = nc.dram_tensor("b1row", [1, 1024], BF, kind="ExternalInput").ap()
    f2_d = nc.dram_tensor("f2", [128, 8, 512], BF, kind="ExternalInput").ap()
    b2row_d = nc.dram_tensor("b2row", [1, 512], BF, kind="ExternalInput").ap()
    f3_d = nc.dram_tensor("f3", [128, 4, 1024], BF, kind="ExternalInput").ap()
    b3row_d = nc.dram_tensor("b3row", [1, 1024], BF, kind="ExternalInput").ap()
    out_d = nc.dram_tensor("out", [BL, 1024], FP, kind="ExternalOutput").ap()

    with tile.TileContext(nc) as tc, ExitStack() as ctx:
        cpool = ctx.enter_context(tc.tile_pool(name="const", bufs=1))
        tail = ctx.enter_context(tc.tile_pool(name="tail", bufs=1))
        psT = ctx.enter_context(tc.tile_pool(name="psT", bufs=1, space="PSUM"))

        identf = cpool.tile([128, 128], FP, tag="identf")
        make_identity(nc, identf[:])
        identb = cpool.tile([128, 128], BF, tag="identb")
        nc.vector.tensor_copy(identb[:], identf[:])
        ones8 = cpool.tile([1, 16], F8, tag="ones8")
        nc.vector.memset(ones8[:], 1.0)
        onesb = cpool.tile([1, 16], BF, tag="onesb")
        nc.vector.memset(onesb[:], 1.0)
        clsTb = cpool.tile([128, ET, 16], BF, tag="clsTb")
        nc.sync.dma_start(clsTb[:], clsTb_d[:])
        qembT = cpool.tile([128, ET, BL, T], BF, tag="qembT")
        goutT = cpool.tile([128, ET, BL], BF, tag="goutT")
        QtT8 = cpool.tile([128, ET, 512], F8, tag="QtT8")
        pcxT = cpool.tile([128, ET, 2 * BL], BF, tag="pcxT")
        wxn_sb = cpool.tile([BL, D], BF, tag="wxn_sb")
        pmaskb = cpool.tile([64, 2], BF, tag="pmaskb")
        nc.sync.dma_start(pmaskb[:], pmask_d[:])

        with tc.tile_pool(name="phW", bufs=1) as phW, \
             tc.tile_pool(name="pgru", bufs=1) as pgru, \
             tc.tile_pool(name="g1", bufs=2) as g1, \
             tc.tile_pool(name="phA", bufs=1) as phA:
            # --- critical-path DMAs first: phase A operands
            clsT8 = phA.tile([128, ET, 16], F8, tag="clsT8")
            nc.sync.dma_start(clsT8[:], clsT8_d[:])
            mw8 = phA.tile([128, ET, G], F8, tag="mw8")
            for tt in range(ET):
                nc.sync.dma_start(mw8[:, tt, :], mw8_d[:, tt, :])
            cw8 = phA.tile([1, G], F8, tag="cw8")
            nc.sync.dma_start(cw8[:], cw8_d[:])
            # --- GRU weights (needed ~10us in)
            wh8 = pgru.tile([128, ET, G], F8, tag="wh8")
            for tt in range(ET):
                nc.sync.dma_start(wh8[:, tt, :], wh8_d[:, tt, :])
            idext8 = pgru.tile([128, 2, 16], F8, tag="idext8")
            nc.sync.dma_start(idext8[:], idext8_d[:])
            ext8 = pgru.tile([128, 2, G], F8, tag="ext8")
            nc.vector.memset(ext8[:].rearrange("p a b -> p (a b)"), 0.0)
            nc.sync.dma_start(ext8[0:BL, 0, 2 * D:3 * D], extn8_d[:])
            # --- GRU initial state (needed immediately after phase A)
            hT8s = [pgru.tile([128, ET, 16], F8, tag=f"hT8{i}",
                              name=f"hT8{i}") for i in range(2)]
            nc.sync.dma_start(hT8s[0][:], h0t8_d[:])
            hnat0 = g1.tile([BL, D], BF, tag="hnat")
            nc.sync.dma_start(hnat0[:], h0b_d[:])
            # --- mid/late-phase weight tiles (DMAs emitted after phase A)
            mg = phW.tile([128, ET, D], BF, tag="mg")
            cgcol = phW.tile([128, ET], FP, tag="cgcol")
            w0 = phW.tile([128, ET, D], BF, tag="w0")
            b0col = phW.tile([128, ET], FP, tag="b0col")
            w1tt = phW.tile([128, ET, D], BF, tag="w1t")
            m2 = tail.tile([128, 2 * ET, D], BF, tag="m2")
            vconcol = tail.tile([128, ET], FP, tag="vconcol")
            f1 = tail.tile([128, 12, 1024], BF, tag="f1")
            f2 = tail.tile([128, 8, 512], BF, tag="f2")
            f3 = tail.tile([128, 4, 1024], BF, tag="f3")
            b1row = tail.tile([1, 1024], BF, tag="b1row")
            b2row = tail.tile([1, 512], BF, tag="b2row")
            b3row = tail.tile([1, 1024], BF, tag="b3row")

            with tc.tile_pool(name="psG", bufs=1, space="PSUM") as psG:
                # ============ phase A: wx = cls@MW + cw ====================

                def gate_mms(lhsT, rhs_w, prz, pn):
                    """emit the 20 DR matmuls for one full [8, 2304] gate set"""
                    subs = [(prz, 0, 512, 0), (prz, 512, 512, 512),
                            (prz, 1024, 512, 1024), (pn, 0, 512, 1536),
                            (pn, 512, 256, 2048)]
                    out = []
                    for (dst, o0, w_, j0) in subs:
                        for g in range(3):
                            nc.tensor.matmul(dst[:, o0:o0 + w_],
                                             lhsT[:, 2 * g:2 * g + 2, :],
                                             rhs_w[:, 2 * g:2 * g + 2, j0:j0 + w_],
                                             start=(g == 0), stop=False,
                                             perf_mode=DR)
                        out.append((dst, o0, w_, j0))
                    return out

                przA = psG.tile([16, 1536], FP, tag="prz")
                pnA = psG.tile([16, 768], FP, tag="pn")
                for (dst, o0, w_, j0) in gate_mms(clsT8, mw8, przA, pnA):
                    nc.tensor.matmul(dst[:, o0:o0 + w_], ones8[:1, :],
                                     cw8[:, j0:j0 + w_], start=False, stop=True)
                nc.scalar.activation(ext8[0:BL, 0, 0:1536], przA[:BL, :],
                                     AF.Copy)
                nc.scalar.activation(wxn_sb[:], pnA[:BL, :], AF.Copy)


                # ============ phase B: GRU =================================
                nc.gpsimd.memset(hT8s[1][:, :, BL:16], 0.0)
                hnat = hnat0

                SUBS = [(0, 0, 512, 0), (0, 512, 512, 512),
                        (0, 1024, 512, 1024), (1, 0, 512, 1536),
                        (1, 512, 256, 2048)]

                def emit_ext(prz_, pn_):
                    """pre-fill next step's psum with the wx/bias rows; no
                    dependency on the recurrent state, so these fill the PE
                    idle gap during the gate-math tail"""
                    for (ti, o0, w__, j0) in SUBS:
                        dst = prz_ if ti == 0 else pn_
                        nc.tensor.matmul(dst[:, o0:o0 + w__], idext8[:],
                                         ext8[:, :, j0:j0 + w__],
                                         start=True, stop=False, perf_mode=DR)

                prz = psG.tile([16, 1536], FP, tag="prz", name="prz_p0")
                pn = psG.tile([16, 768], FP, tag="pn", name="pn_p0")
                emit_ext(prz, pn)

                for t in range(KSTEPS):
                    hT8 = hT8s[t % 2]
                    hT8n = hT8s[(t + 1) % 2]
                    subs = [(prz, 0, 512, 0), (prz, 512, 512, 512),
                            (prz, 1024, 512, 1024), (pn, 0, 512, 1536),
                            (pn, 512, 256, 2048)]
                    rz = g1.tile([BL, 2 * D], BF, tag="rz")
                    u_ = g1.tile([BL, D], BF, tag="u")
                    w_ = g1.tile([BL, D], BF, tag="w")
                    t1 = g1.tile([BL, D], BF, tag="t1")
                    t2 = g1.tile([BL, D], BF, tag="t2")
                    nt_ = g1.tile([BL, D], BF, tag="nt")
                    v_ = g1.tile([BL, D], BF, tag="v")
                    hnew = g1.tile([BL, D], BF, tag="hnat")

                    def sub_mms(si):
                        dst, o0, w__, j0 = subs[si]
                        for g in range(3):
                            nc.tensor.matmul(dst[:, o0:o0 + w__],
                                             hT8[:, 2 * g:2 * g + 2, :],
                                             wh8[:, 2 * g:2 * g + 2, j0:j0 + w__],
                                             start=False, stop=(g == 2),
                                             perf_mode=DR)

                    sub_mms(0)
                    sub_mms(1)
                    # r ready -> sigmoid overlaps remaining stream
                    nc.scalar.activation(rz[:, 0:D], prz[:BL, 0:D], AF.Sigmoid,
                                         scale=1.0 / SG)
                    sub_mms(2)
                    nc.scalar.activation(rz[:, D:2 * D], prz[:BL, D:2 * D],
                                         AF.Sigmoid, scale=1.0 / SG)
                    # z-dependent terms during the n-chunk stream
                    nc.vector.tensor_mul(u_[:], rz[:, D:2 * D], hnat[:])
                    nc.vector.tensor_scalar(w_[:], rz[:, D:2 * D], -1.0, 1.0,
                                            OP.mult, OP.add)
                    sub_mms(3)
                    nc.vector.tensor_mul(t1[:, 0:512], rz[:, 0:512],
                                         pn[:BL, 0:512])
                    nc.vector.tensor_add(t2[:, 0:512], t1[:, 0:512],
                                         wxn_sb[:, 0:512])
                    nc.scalar.activation(nt_[:, 0:512], t2[:, 0:512], AF.Tanh,
                                         scale=1.0 / SG)
                    sub_mms(4)
                    for dd in range(NDUM):
                        pdum = psT.tile([16, 512], FP, tag="pt",
                                        padded_shape=[128, 1024])
                        nc.tensor.matmul(pdum[:], idext8[:],
                                         ext8[:, :, 0:512], start=True,
                                         stop=True, perf_mode=DR)
                    nc.vector.tensor_mul(t1[:, 512:768], rz[:, 512:768],
                                         pn[:BL, 512:768])
                    nc.vector.tensor_add(t2[:, 512:768], t1[:, 512:768],
                                         wxn_sb[:, 512:768])
                    # first half of h' only needs tanh_a: runs under the
                    # s4 matmuls / ext-prefill window
                    nc.vector.tensor_mul(v_[:, 0:512], w_[:, 0:512],
                                         nt_[:, 0:512])
                    nc.vector.tensor_add(hnew[:, 0:512], v_[:, 0:512],
                                         u_[:, 0:512])
                    nc.scalar.activation(nt_[:, 512:768], t2[:, 512:768],
                                         AF.Tanh, scale=1.0 / SG)
                    if t < KSTEPS - 1:
                        prz = psG.tile([16, 1536], FP, tag="prz",
                                       name=f"prz_p{t + 1}")
                        pn = psG.tile([16, 768], FP, tag="pn",
                                      name=f"pn_p{t + 1}")
                        emit_ext(prz, pn)
                    pt = psT.tile([128, 64], BF, tag="pt",
                                  padded_shape=[128, 1024])
                    for kt in range(4):
                        nc.tensor.matmul(pt[:, 8 * kt:8 * kt + 8],
                                         hnew[:, 128 * kt:128 * (kt + 1)],
                                         identb[:BL, :BL], is_transpose=True,
                                         skip_group_check=True)
                    nc.vector.tensor_copy(
                        hT8n[:, 0:4, 0:BL],
                        pt[:, :32].rearrange("p (a b) -> p a b", b=BL))
                    nc.vector.tensor_mul(v_[:, 512:768], w_[:, 512:768],
                                         nt_[:, 512:768])
                    nc.vector.tensor_add(hnew[:, 512:768], v_[:, 512:768],
                                         u_[:, 512:768])
                    for kt in range(4, ET):
                        nc.tensor.matmul(pt[:, 8 * kt:8 * kt + 8],
                                         hnew[:, 128 * kt:128 * (kt + 1)],
                                         identb[:BL, :BL], is_transpose=True,
                                         skip_group_check=True)
                    nc.vector.tensor_copy(
                        hT8n[:, 4:ET, 0:BL],
                        pt[:, 32:48].rearrange("p (a b) -> p a b", b=BL))
                    nc.scalar.copy(
                        qembT[:, :, :, t].rearrange("p a b -> p (a b)"),
                        pt[:, :48])
                    hnat = hnew

                    if t == 14:
                        gate1 = g1.tile([1, BL], BF, tag="gate1")
                        nc.sync.dma_start(gate1[:], qembT[0:1, 0, :, 14])
                        for tt in range(0, ET, 3):
                            nc.sync.dma_start(mg[:, tt:tt + 3, :],
                                              mg_d[:, tt:tt + 3, :])
                        nc.sync.dma_start(cgcol[:], cgcol_d[:])
                        for tt in range(0, ET, 3):
                            nc.sync.dma_start(w0[:, tt:tt + 3, :],
                                              w0_d[:, tt:tt + 3, :])
                        nc.sync.dma_start(b0col[:], b0col_d[:])
                        for tt in range(0, ET, 3):
                            nc.sync.dma_start(w1tt[:, tt:tt + 3, :],
                                              w1t_d[:, tt:tt + 3, :])
                    if t == 22:
                        gate2 = g1.tile([1, BL], BF, tag="gate2")
                        nc.sync.dma_start(gate2[:], qembT[0:1, 0, :, 22])
                        for tt in range(0, 2 * ET, 3):
                            nc.sync.dma_start(m2[:, tt:tt + 3, :],
                                              m2_d[:, tt:tt + 3, :])
                        nc.sync.dma_start(vconcol[:], vconcol_d[:])
                        for tt in range(0, 12, 3):
                            nc.sync.dma_start(f1[:, tt:tt + 3, :],
                                              f1_d[:, tt:tt + 3, :])
                        for tt in range(0, 8, 4):
                            nc.sync.dma_start(f2[:, tt:tt + 4, :],
                                              f2_d[:, tt:tt + 4, :])
                        for tt in range(0, 4, 2):
                            nc.sync.dma_start(f3[:, tt:tt + 2, :],
                                              f3_d[:, tt:tt + 2, :])
                        nc.sync.dma_start(b1row[:], b1row_d[:])
                        nc.sync.dma_start(b2row[:], b2row_d[:])
                        nc.sync.dma_start(b3row[:], b3row_d[:])

            # ============ gout = cls@Mg + cg (feature-major) ==============
            with tc.tile_pool(name="psC", bufs=2, space="PSUM") as psC:
                for mt in range(ET):
                    p = psC.tile([128, BL], FP, tag="pg",
                                 padded_shape=[128, 512])
                    for kt in range(ET):
                        nc.tensor.matmul(p[:],
                                         mg[:, kt, 128 * mt:128 * (mt + 1)],
                                         clsTb[:, kt, 0:BL],
                                         start=(kt == 0), stop=(kt == ET - 1))
                    nc.vector.tensor_scalar(goutT[:, mt, :], p[:],
                                            cgcol[:, mt:mt + 1], None, OP.add)

                # ============ phase C: QT, QtT8 ===========================
                if PHASES >= 2:
                    qflat = qembT[:].rearrange("p a b t -> p a (b t)")
                    QT = g1.tile([128, ET, BL * T], BF, tag="QT")
                    for mt in range(ET):
                        p = psC.tile([128, BL * T], FP, tag="pc",
                                     padded_shape=[128, 512])
                        for kt in range(ET):
                            nc.tensor.matmul(
                                p[:], w0[:, kt, 128 * mt:128 * (mt + 1)],
                                qflat[:, kt, :],
                                start=(kt == 0), stop=(kt == ET - 1))
                        nc.vector.tensor_scalar(QT[:, mt, :], p[:],
                                                b0col[:, mt:mt + 1], None,
                                                OP.add)
                    for hd in range(NH):
                        for mt in range(ET):
                            p = psC.tile([128, BL * T], FP, tag="pc",
                                         padded_shape=[128, 512])
                            for i in range(3):
                                kt = 3 * hd + i
                                nc.tensor.matmul(
                                    p[:], w1tt[:, kt, 128 * mt:128 * (mt + 1)],
                                    QT[:, kt, :],
                                    start=(i == 0), stop=(i == 2))
                            dst = QtT8[:, mt, :].rearrange(
                                "p (b h2 t) -> p b h2 t", h2=NH, t=T)[:, :, hd, :]
                            src = p[:].rearrange("p (b t) -> p b t", t=T)
                            nc.scalar.activation(dst, src, AF.Copy, scale=SQ)

        # ================= phase D: per-b attention (sw-pipelined) ========
        if PHASES >= 3:
            with tc.tile_pool(name="xb", bufs=3) as xb, \
                 tc.tile_pool(name="ab", bufs=2) as ab, \
                 tc.tile_pool(name="psS", bufs=2, space="PSUM") as psS:
                state = {}
                gate3 = ab.tile([1, BL], BF, tag="gate3")
                nc.gpsimd.dma_start(gate3[:], qembT[0:1, 0, :, 24])

                def emit_scores(b):
                    xn_t = xb.tile([128, 8, D], BF, tag="xn")
                    nc.gpsimd.memset(xn_t[:, 7, :], 0.0)
                    xsrc = xn_d[b, 0:896, :].rearrange("(c p) d -> p c d", p=128)
                    nc.gpsimd.dma_start(xn_t[:, 0:4, :], xsrc[:, 0:4, :])
                    nc.gpsimd.dma_start(xn_t[:, 4:7, :], xsrc[:, 4:7, :])
                    nc.gpsimd.dma_start(xn_t[:4, 7, :], xn_d[b, 896:900, :])
                    xt_t = xb.tile([128, ET, 912], F8, tag="xt")
                    tsrc = xt8_d[b, :, :].rearrange("(c p) n -> p c n", p=128)
                    nc.gpsimd.dma_start(xt_t[:, 0:3, :], tsrc[:, 0:3, :])
                    nc.gpsimd.dma_start(xt_t[:, 3:6, :], tsrc[:, 3:6, :])
                    att = ab.tile([64, NK], BF, tag="att")
                    zacc = ab.tile([64, 2], FP, tag="zacc")
                    for ci, (n0, nw) in enumerate(CH_NK):
                        p = psS.tile([64, 512], FP, tag="s")
                        for g in range(3):
                            nc.tensor.matmul(
                                p[:, :nw],
                                QtT8[:, 2 * g:2 * g + 2, 64 * b:64 * b + 64],
                                xt_t[:, 2 * g:2 * g + 2, n0:n0 + nw],
                                start=(g == 0), stop=(g == 2), perf_mode=DR)
                        nc.scalar.activation(att[:, n0:n0 + nw], p[:, :nw],
                                             AF.Exp, scale=ESC,
                                             accum_out=zacc[:, ci:ci + 1])
                    state[b] = (xn_t, att, zacc)

                def emit_tail(b):
                    xn_t, att, zacc = state.pop(b)
                    zs = ab.tile([64, 1], FP, tag="zs")
                    nc.vector.tensor_add(zs[:], zacc[:, 0:1], zacc[:, 1:2])
                    rcp = ab.tile([64, 1], FP, tag="rcp")
                    nc.vector.reciprocal(rcp[:], zs[:])
                    wm = ab.tile([64, 2], BF, tag="wm")
                    nc.vector.tensor_scalar(wm[:], pmaskb[:], rcp[:, 0:1],
                                            None, OP.mult)
                    pa_sb = ab.tile([2, NK], BF, tag="pa_sb")
                    for ci, (n0, nw) in enumerate(CH_NK):
                        p2 = psS.tile([2, 512], FP, tag="p2")
                        nc.tensor.matmul(p2[:, :nw], wm[:], att[:, n0:n0 + nw],
                                         start=True, stop=True)
                        nc.vector.tensor_copy(pa_sb[:, n0:n0 + nw],
                                              p2[:, :nw])
                    paT = ab.tile([128, len(KC), 2], BF, tag="paT")
                    nc.gpsimd.memset(paT[:].rearrange("p a b -> p (a b)"), 0.0)
                    ptp = psT.tile([128, 16], BF, tag="pt",
                                   padded_shape=[128, 1024])
                    for c, (k0, kw) in enumerate(KC):
                        nc.tensor.matmul(ptp[:kw, 2 * c:2 * c + 2],
                                         pa_sb[:, k0:k0 + kw], identb[:2, :2],
                                         is_transpose=True,
                                         skip_group_check=True)
                        nc.vector.tensor_copy(paT[:kw, c, :],
                                              ptp[:kw, 2 * c:2 * c + 2])
                    pcx = ab.tile([2, D], BF, tag="pcx")
                    for ci, (n0, nw) in enumerate(CH_D):
                        p = psS.tile([2, 512], FP, tag="v")
                        for c in range(len(KC)):
                            nc.tensor.matmul(p[:, :nw], paT[:, c, :],
                                             xn_t[:, c, n0:n0 + nw],
                                             start=(c == 0),
                                             stop=(c == len(KC) - 1))
                        nc.vector.tensor_copy(pcx[:, n0:n0 + nw], p[:, :nw])
                    for dd in range(2):
                        pdum = psT.tile([64, 512], FP, tag="pt",
                                        padded_shape=[128, 1024])
                        nc.tensor.matmul(
                            pdum[:], QtT8[:, 0:2, 0:64],
                            QtT8[:, 0:2, 0:512],
                            start=True, stop=True, perf_mode=DR)
                    ptc = psT.tile([128, 16], BF, tag="pt",
                                   padded_shape=[128, 1024])
                    for kt in range(ET):
                        nc.tensor.matmul(ptc[:, 2 * kt:2 * kt + 2],
                                         pcx[:, 128 * kt:128 * (kt + 1)],
                                         identb[:2, :2], is_transpose=True,
                                         skip_group_check=True)
                    nc.vector.tensor_copy(
                        pcxT[:, :, 2 * b:2 * b + 2],
                        ptc[:, :2 * ET].rearrange("p (a c) -> p a c", c=2))

                for b in range(BL):
                    emit_scores(b)
                    if b > 0:
                        emit_tail(b - 1)
                emit_tail(BL - 1)

        # ================= phase E: projections + MLP =====================
        if PHASES >= 4:
            with tc.tile_pool(name="psE", bufs=2, space="PSUM") as psE:
                loT = tail.tile([128, ET, BL], BF, tag="loT")
                for mt in range(ET):
                    p = psE.tile([128, BL], FP, tag="pe",
                                 padded_shape=[128, 512])
                    k = 0
                    for hd in range(NH):
                        for kt in range(ET):
                            pcv = pcxT[:].rearrange(
                                "p a (b h) -> p a b h", h=2)[:, kt, :, hd]
                            nc.tensor.matmul(
                                p[:], m2[:, ET * hd + kt, 128 * mt:128 * (mt + 1)],
                                pcv, start=(k == 0), stop=(k == 2 * ET - 1))
                            k += 1
                    nc.vector.tensor_scalar(loT[:, mt, :], p[:],
                                            vconcol[:, mt:mt + 1], None,
                                            OP.add)

                y1b = tail.tile([BL, 1024], BF, tag="y1b")
                for ch in range(2):
                    p = psE.tile([BL, 512], FP, tag="pe")
                    for kt in range(12):
                        lhs = loT[:, kt, :] if kt < ET else goutT[:, kt - ET, :]
                        nc.tensor.matmul(p[:], lhs,
                                         f1[:, kt, 512 * ch:512 * (ch + 1)],
                                         start=(kt == 0), stop=False)
                    nc.tensor.matmul(p[:], onesb[:1, :BL],
                                     b1row[:, 512 * ch:512 * (ch + 1)],
                                     start=False, stop=True)
                    nc.scalar.activation(y1b[:, 512 * ch:512 * (ch + 1)], p[:],
                                         AF.Copy)
                pt1 = psT.tile([128, 64], BF, tag="pt",
                               padded_shape=[128, 1024])
                for kt in range(8):
                    nc.tensor.matmul(pt1[:, 8 * kt:8 * kt + 8],
                                     y1b[:, 128 * kt:128 * (kt + 1)],
                                     identb[:BL, :BL], is_transpose=True,
                                     skip_group_check=True)
                y1T = tail.tile([128, 8, BL], BF, tag="y1T")
                nc.vector.tensor_copy(y1T[:].rearrange("p a b -> p (a b)"),
                                      pt1[:, :64])

                p = psE.tile([BL, 512], FP, tag="pe")
                for kt in range(8):
                    nc.tensor.matmul(p[:], y1T[:, kt, :], f2[:, kt, :],
                                     start=(kt == 0), stop=False)
                nc.tensor.matmul(p[:], onesb[:1, :BL], b2row[:],
                                 start=False, stop=True)
                y2b = tail.tile([BL, 512], BF, tag="y2b")
                nc.scalar.activation(y2b[:], p[:], AF.Relu)
                pt2 = psT.tile([128, 32], BF, tag="pt",
                               padded_shape=[128, 1024])
                for kt in range(4):
                    nc.tensor.matmul(pt2[:, 8 * kt:8 * kt + 8],
                                     y2b[:, 128 * kt:128 * (kt + 1)],
                                     identb[:BL, :BL], is_transpose=True,
                                     skip_group_check=True)
                y2T = tail.tile([128, 4, BL], BF, tag="y2T")
                nc.vector.tensor_copy(y2T[:].rearrange("p a b -> p (a b)"),
                                      pt2[:, :32])

                ynat = tail.tile([BL, 1024], FP, tag="ynat")
                for ch in range(2):
                    p = psE.tile([BL, 512], FP, tag="pe")
                    for kt in range(4):
                        nc.tensor.matmul(p[:], y2T[:, kt, :],
                                         f3[:, kt, 512 * ch:512 * (ch + 1)],
                                         start=(kt == 0), stop=False)
                    nc.tensor.matmul(p[:], onesb[:1, :BL],
                                     b3row[:, 512 * ch:512 * (ch + 1)],
                                     start=False, stop=True)
                    nc.vector.tensor_copy(ynat[:, 512 * ch:512 * (ch + 1)],
                                          p[:])
                nc.sync.dma_start(out_d[:, :], ynat[:])

    nc.compile()
    return nc


_NC = None


def _bf(x):
    return np.ascontiguousarray(x).astype(ml_dtypes.bfloat16)


def _f8(x):
    return np.ascontiguousarray(x).astype(ml_dtypes.float8_e4m3)


def _tile6(w):
    """[768, J] -> [128, 6, J] with [p, t, j] = w[128t+p, j]"""
    J = w.shape[1]
    return np.ascontiguousarray(w.reshape(ET, 128, J).transpose(1, 0, 2))


def make_in_maps(inputs):
    f32 = np.float32
    img = np.asarray(inputs["image_local_embeds"], f32)
    h0 = np.asarray(inputs["h0"], f32)
    w_ih = np.asarray(inputs["gru_w_ih"], f32)
    w_hh = np.asarray(inputs["gru_w_hh"], f32)
    b_ih = np.asarray(inputs["gru_b_ih"], f32)
    b_hh = np.asarray(inputs["gru_b_hh"], f32)
    ga_w = np.asarray(inputs["ga_w"], f32)
    ga_b = np.asarray(inputs["ga_b"], f32)
    ga_pool = np.asarray(inputs["ga_pool"], f32)
    la_w = np.asarray(inputs["la_w"], f32)
    la_b = np.asarray(inputs["la_b"], f32)
    la_pool = np.asarray(inputs["la_pool"], f32)
    go_w = np.asarray(inputs["go_w"], f32)
    go_b = np.asarray(inputs["go_b"], f32)
    go_pool = np.asarray(inputs["go_pool"], f32)
    f1_w = np.asarray(inputs["f1_w"], f32)
    f1_b = np.asarray(inputs["f1_b"], f32)
    f2_w = np.asarray(inputs["f2_w"], f32)
    f2_b = np.asarray(inputs["f2_b"], f32)
    f3_w = np.asarray(inputs["f3_w"], f32)
    f3_b = np.asarray(inputs["f3_b"], f32)

    Mga = ga_pool[0] * (ga_w[2] @ ga_w[3])
    cga = ga_pool[0] * (ga_b[2] @ ga_w[3] + ga_b[3])
    MW = Mga @ w_ih.T
    cw = cga @ w_ih.T + b_ih
    cw[:2 * D] += b_hh[:2 * D]
    Sgo = go_pool.sum()
    Mg = Sgo * (go_w[2] @ go_w[3])
    cg = Sgo * (go_b[2] @ go_w[3] + go_b[3])
    Sla = la_pool.sum()
    M2 = np.stack([la_w[2][:, hd * DK:(hd + 1) * DK]
                   @ la_w[3][hd * DK:(hd + 1) * DK, :] for hd in range(NH)])
    vcon = Sla * (la_b[2] @ la_w[3] + la_b[3])
    W1T = np.ascontiguousarray(la_w[1].T)

    idext = np.zeros((128, 2, 16), f32)
    for b in range(BL):
        idext[b, 0, b] = 1.0
    pmask = np.zeros((64, 2), f32)
    pmask[0:T, 0] = la_pool
    pmask[T:2 * T, 1] = la_pool

    mw8 = _f8(_tile6(SG * MW))
    cw8 = _f8((SG * cw)[None, :])
    wh8 = _f8(_tile6(SG * w_hh.T))
    extn8 = _f8(np.broadcast_to(SG * b_hh[2 * D:], (BL, D)).copy())
    idext8 = _f8(idext)
    mg = _bf(_tile6(Mg))
    cgcol = np.ascontiguousarray(cg.reshape(ET, 128).T)
    w0 = _bf(_tile6(la_w[0]))
    b0col = np.ascontiguousarray(la_b[0].reshape(ET, 128).T)
    w1t = _bf(_tile6(W1T))
    m2 = _bf(np.concatenate([_tile6(M2[0]), _tile6(M2[1])], axis=1))
    vconcol = np.ascontiguousarray(vcon.reshape(ET, 128).T)
    f1p = _bf(f1_w.reshape(12, 128, 1024).transpose(1, 0, 2))
    f2p = _bf(f2_w.reshape(8, 128, 512).transpose(1, 0, 2))
    f3p = _bf(f3_w.reshape(4, 128, 1024).transpose(1, 0, 2))

    in_maps = []
    B = img.shape[0]
    per = B // NCORES
    for c in range(NCORES):
        sl = slice(c * per, (c + 1) * per)
        cls = img[sl, 0, :]
        X = img[sl, 1:, :]
        clsT = np.zeros((128, ET, 16), f32)
        clsT[:, :, :BL] = cls.T.reshape(ET, 128, BL).transpose(1, 0, 2)
        h0c = h0[sl]
        h0t = np.zeros((128, ET, 16), f32)
        h0t[:, :, :BL] = h0c.T.reshape(ET, 128, BL).transpose(1, 0, 2)
        xt = np.zeros((per, D, 912), f32)
        xt[:, :, :NK] = SX * X.transpose(0, 2, 1)
        m = {
            "clsT8": _f8(clsT),
            "clsTb": _bf(clsT),
            "mw8": mw8, "cw8": cw8, "idext8": idext8, "extn8": extn8,
            "wh8": wh8,
            "h0t8": _f8(h0t), "h0b": _bf(h0c),
            "mg": mg, "cgcol": cgcol.astype(f32),
            "w0": w0, "b0col": b0col.astype(f32),
            "w1t": w1t, "pmaskb": _bf(pmask),
            "xn": _bf(X), "xt8": _f8(xt),
            "m2": m2, "vconcol": vconcol.astype(f32),
            "f1": f1p, "b1row": _bf(f1_b[None, :]),
            "f2": f2p, "b2row": _bf(f2_b[None, :]),
            "f3": f3p, "b3row": _bf(f3_b[None, :]),
        }
        in_maps.append(m)
    return in_maps


def kernel(**inputs):
    global _NC
    if _NC is None:
        _NC = build()
    in_maps = make_in_maps(inputs)
    res = run_bass_kernel_spmd(_NC, in_maps, core_ids=list(range(NCORES)))
    return np.concatenate([res.results[c]["out"] for c in range(NCORES)],
                          axis=0)
